# revision 19
# baseline (speedup 1.0000x reference)
"""BiologicallyInformedLoss Trainium2 kernel (v2).

Data-parallel over batch: 64 sequences -> 8 NeuronCores x 8 sequences.

Per-core layout (one chunk == one sequence): position n = p*64 + k with
p = partition (0..127), k = 0..63.  Within a partition row the logits for
a chunk are stored c-major / k-inner: free offset = c*64 + k.  The host
pre-permutes and casts everything, so every device DMA is a contiguous
[128, N] block.

Device work per chunk (the heavy part):
  - ScalarE: exp(x) (bf16), ln(se) -> lse
  - VectorE: max tree over c (7 tensor_tensor max ops, bf16 2x mode),
    one-hot via is_ge against the per-position max broadcast over the
    MIDDLE axis (innermost stays stride-1 so the 2x perf mode applies),
    v-weighted lse reduction
  - TensorE: sum-over-c of exp via 65 accumulating identity matmuls into
    PSUM (f32 accumulation), per-sequence histograms via 64 accumulating
    [128,2]x[128,65] matmuls
Host: exact x_t gather for the CE numerator, target histograms,
CAI/RSCU/KL finalization on 65-wide vectors, final weighted sum.
"""
import sys
import numpy as np

sys.path.insert(0, "/opt/trn_rl_repo/concourse")
sys.path.insert(0, "/opt/trn_rl_repo")

import ml_dtypes  # noqa: E402

BF16 = ml_dtypes.bfloat16

# ---- problem constants (mirrors reference.py; hardcoded) ----
AA64 = "FFLLSSSSYY**CC*WLLLLPPPPHHQQRRRRIIIMTTTTNNKKSSRRVVVVAAAADDEEGGGG"
NC_ = 65
_uniq = sorted(set(AA64))
_gid = {a: i + 1 for i, a in enumerate(_uniq)}
NG = len(_uniq) + 1
GROUP_IDS = np.array([0] + [_gid[a] for a in AA64], dtype=np.int32)
IS_CODING = np.array([False] + [a != "*" for a in AA64])
_syn = {a: AA64.count(a) for a in _uniq}
NSYN = np.array([0.0] + [float(_syn[a]) for a in AA64], dtype=np.float32)
LOSS_W = dict(ce=1.0, cai=0.4, rscu=0.3, gc=0.1, structure=0.15, dynamics=0.1)
EPS = 1e-8

B, L = 64, 8192
NCORES = 8
SEQ_PER_CORE = B // NCORES          # 8
P = 128                             # partitions
KC = 64                             # positions per partition per chunk
NCHUNK = SEQ_PER_CORE               # 8 chunks == 8 sequences
CW = NC_ * KC                       # 4160 free elements per chunk

_BASS_CACHE = {}


def _build_bass(n_pe_se=0, repeat=1, se_l1pe=True, ident_f8=False, pair=False):
    """n_pe_se: how many of the 8 chunks compute sum-of-exp fully on the PE
    (accumulating identity matmuls) instead of the DVE bf16 tree.
    se_l1pe: all chunks do one DVE add level (65->33 slices) and the PE
    finishes the sum with 33 accumulating identity matmuls — balances the
    two engines.  ident_f8: use an fp8 identity (cheaper LDWEIGHTS).
    pair: process chunks in pairs with fused DVE/ACT ops (halves the
    per-op overheads of the trees / is_ge / exp)."""
    import concourse.bacc as bacc
    import concourse.tile as tile
    import concourse.mybir as mybir

    f32 = mybir.dt.float32
    bf16 = mybir.dt.bfloat16
    Alu = mybir.AluOpType
    Act = mybir.ActivationFunctionType
    Ax = mybir.AxisListType

    nc = bacc.Bacc(None, target_bir_lowering=False)

    f8 = mybir.dt.float8e4
    xb = nc.declare_dram_parameter("xb", [P, NCHUNK, CW], bf16, isOutput=False)
    mb_in = nc.declare_dram_parameter("mb", [P, NCHUNK, KC, 2], bf16, isOutput=False)
    vt_in = nc.declare_dram_parameter("vt", [P, NCHUNK * KC], bf16, isOutput=False)
    gp_in = nc.declare_dram_parameter("gpp", [P, 2 * NCHUNK * KC], f32, isOutput=False)
    id_in = nc.declare_dram_parameter(
        "identf8" if ident_f8 else "ident", [P, P], f8 if ident_f8 else bf16,
        isOutput=False)
    tok_in = nc.declare_dram_parameter("tok", [16, 16], f32, isOutput=False)

    lse_out = nc.declare_dram_parameter("lse_acc", [P, NCHUNK], f32, isOutput=True)
    hist_out = nc.declare_dram_parameter("hist", [2, NCHUNK, NC_], f32, isOutput=True)
    gps_out = nc.declare_dram_parameter("gps", [P, 2 * NCHUNK], f32, isOutput=True)
    tok_out = nc.declare_dram_parameter("tok_out", [16, 16], f32, isOutput=True)

    with tile.TileContext(nc) as tc:
        with tc.tile_pool(name="one", bufs=1) as one, \
             tc.tile_pool(name="xp", bufs=3) as xp, \
             tc.tile_pool(name="ep", bufs=2) as ep, \
             tc.tile_pool(name="qp", bufs=2) as qp, \
             tc.tile_pool(name="tp", bufs=2) as tp, \
             tc.tile_pool(name="pse", bufs=2, space="PSUM") as pse, \
             tc.tile_pool(name="ph", bufs=2, space="PSUM") as ph:

            def body(_iv=None):
                tok_sb = one.tile([16, 16], f32, tag="tok_sb")
                nc.sync.dma_start(out=tok_sb, in_=tok_in[:])
                nc.sync.dma_start(out=tok_out[:], in_=tok_sb)

                ident = one.tile([P, P], f8 if ident_f8 else bf16, tag="ident")
                nc.sync.dma_start(out=ident, in_=id_in[:])
                vt = one.tile([P, NCHUNK * KC], bf16, tag="vt")
                nc.sync.dma_start(out=vt, in_=vt_in[:])
                mbt = one.tile([P, NCHUNK, KC, 2], bf16, tag="mbt")
                nc.sync.dma_start(out=mbt, in_=mb_in[:])
                gpt = one.tile([P, 2 * NCHUNK * KC], f32, tag="gpt")
                nc.sync.dma_start(out=gpt, in_=gp_in[:])

                gps_sb = one.tile([P, 2 * NCHUNK], f32, tag="gps_sb")
                nc.vector.tensor_reduce(
                    gps_sb[:].rearrange("p (t s) -> p t s", t=2),
                    gpt[:].rearrange("p (t s k) -> p t s k", t=2, s=NCHUNK),
                    Ax.X, Alu.add)

                lse_sb = one.tile([P, NCHUNK], f32, tag="lse_sb")
                hist_sb = one.tile([2, NCHUNK, NC_], f32, tag="hist_sb")

                if pair:
                    for j in range(NCHUNK // 2):
                        x2 = xp.tile([P, 2, NC_, KC], bf16, tag="x")
                        nc.sync.dma_start(
                            out=x2[:].rearrange("p u c k -> p (u c k)"),
                            in_=xb[:, 2 * j:2 * j + 2, :].rearrange(
                                "p u w -> p (u w)"))
                        ex2 = ep.tile([P, 2, NC_, KC], bf16, tag="ex")
                        nc.scalar.activation(
                            ex2[:].rearrange("p u c k -> p (u c k)"),
                            x2[:].rearrange("p u c k -> p (u c k)"), Act.Exp)
                        m32 = tp.tile([P, 2, 32, KC], bf16, tag="m32")
                        nc.vector.tensor_tensor(m32[:], x2[:, :, 0:32, :],
                                                x2[:, :, 32:64, :], Alu.max)
                        m16 = tp.tile([P, 2, 16, KC], bf16, tag="m16")
                        nc.vector.tensor_tensor(m16[:], m32[:, :, 0:16, :],
                                                m32[:, :, 16:32, :], Alu.max)
                        m8 = tp.tile([P, 2, 8, KC], bf16, tag="m8")
                        nc.vector.tensor_tensor(m8[:], m16[:, :, 0:8, :],
                                                m16[:, :, 8:16, :], Alu.max)
                        m4 = tp.tile([P, 2, 4, KC], bf16, tag="m4")
                        nc.vector.tensor_tensor(m4[:], m8[:, :, 0:4, :],
                                                m8[:, :, 4:8, :], Alu.max)
                        m2 = tp.tile([P, 2, 2, KC], bf16, tag="m2")
                        nc.vector.tensor_tensor(m2[:], m4[:, :, 0:2, :],
                                                m4[:, :, 2:4, :], Alu.max)
                        m1 = tp.tile([P, 2, 1, KC], bf16, tag="m1")
                        nc.vector.tensor_tensor(m1[:], m2[:, :, 0:1, :],
                                                m2[:, :, 1:2, :], Alu.max)
                        mx = tp.tile([P, 2, 1, KC], bf16, tag="mx")
                        nc.vector.tensor_tensor(mx[:], m1[:],
                                                x2[:, :, 64:65, :], Alu.max)
                        eq2 = qp.tile([P, 2, NC_, KC], bf16, tag="eq")
                        nc.vector.tensor_tensor(
                            eq2[:], x2[:],
                            mx[:].broadcast_to([P, 2, NC_, KC]), Alu.is_ge)
                        s32 = tp.tile([P, 2, 32, KC], bf16, tag="s32")
                        nc.vector.tensor_tensor(s32[:], ex2[:, :, 0:32, :],
                                                ex2[:, :, 32:64, :], Alu.add)
                        for u in range(2):
                            cc = 2 * j + u
                            psum_se = pse.tile([P, KC], f32, tag="psum_se")
                            for c in range(32):
                                nc.tensor.matmul(psum_se[:], ident[:],
                                                 s32[:, u, c, :],
                                                 start=(c == 0), stop=False)
                            nc.tensor.matmul(psum_se[:], ident[:],
                                             ex2[:, u, 64, :],
                                             start=False, stop=True)
                            lse = tp.tile([P, KC], f32, tag="lse")
                            nc.scalar.activation(lse[:], psum_se[:], Act.Ln)
                            junk = tp.tile([P, KC], f32, tag="junk")
                            nc.vector.tensor_tensor(
                                junk[:], lse[:],
                                vt[:, cc * KC:(cc + 1) * KC], Alu.mult)
                            nc.vector.tensor_reduce(lse_sb[:, cc:cc + 1],
                                                    junk[:], Ax.X, Alu.add)
                            psum_h = ph.tile([2, NC_], f32, tag="psum_h")
                            for k in range(KC):
                                nc.tensor.matmul(psum_h[:], mbt[:, cc, k, :],
                                                 eq2[:, u, :, k],
                                                 start=(k == 0),
                                                 stop=(k == KC - 1))
                            nc.scalar.copy(hist_sb[:, cc, :], psum_h[:])

                for cc in ([] if pair else range(NCHUNK)):
                    x = xp.tile([P, NC_, KC], bf16, tag="x")
                    nc.sync.dma_start(
                        out=x[:].rearrange("p c k -> p (c k)"),
                        in_=xb[:, cc, :])

                    ex = ep.tile([P, NC_, KC], bf16, tag="ex")
                    nc.scalar.activation(
                        ex[:].rearrange("p c k -> p (c k)"),
                        x[:].rearrange("p c k -> p (c k)"), Act.Exp)

                    # max tree over c on raw logits (runs parallel to exp)
                    m32 = tp.tile([P, 32, KC], bf16, tag="m32")
                    nc.vector.tensor_tensor(m32[:], x[:, 0:32, :], x[:, 32:64, :], Alu.max)
                    m16 = tp.tile([P, 16, KC], bf16, tag="m16")
                    nc.vector.tensor_tensor(m16[:], m32[:, 0:16, :], m32[:, 16:32, :], Alu.max)
                    m8 = tp.tile([P, 8, KC], bf16, tag="m8")
                    nc.vector.tensor_tensor(m8[:], m16[:, 0:8, :], m16[:, 8:16, :], Alu.max)
                    m4 = tp.tile([P, 4, KC], bf16, tag="m4")
                    nc.vector.tensor_tensor(m4[:], m8[:, 0:4, :], m8[:, 4:8, :], Alu.max)
                    m2 = tp.tile([P, 2, KC], bf16, tag="m2")
                    nc.vector.tensor_tensor(m2[:], m4[:, 0:2, :], m4[:, 2:4, :], Alu.max)
                    m1 = tp.tile([P, 1, KC], bf16, tag="m1")
                    nc.vector.tensor_tensor(m1[:], m2[:, 0:1, :], m2[:, 1:2, :], Alu.max)
                    mx = tp.tile([P, 1, KC], bf16, tag="mx")
                    nc.vector.tensor_tensor(mx[:], m1[:], x[:, 64:65, :], Alu.max)

                    # one-hot (multi-hot on exact bf16 ties), middle-axis
                    # broadcast keeps innermost stride 1 -> DVE 2x mode
                    eq = qp.tile([P, NC_, KC], bf16, tag="eq")
                    nc.vector.tensor_tensor(
                        eq[:], x[:], mx[:].broadcast_to([P, NC_, KC]), Alu.is_ge)

                    # sum over c of exp
                    if se_l1pe:
                        s32 = tp.tile([P, 32, KC], bf16, tag="s32")
                        nc.vector.tensor_tensor(s32[:], ex[:, 0:32, :],
                                                ex[:, 32:64, :], Alu.add)
                        psum_se = pse.tile([P, KC], f32, tag="psum_se")
                        for c in range(32):
                            nc.tensor.matmul(psum_se[:], ident[:], s32[:, c, :],
                                             start=(c == 0), stop=False)
                        nc.tensor.matmul(psum_se[:], ident[:], ex[:, 64, :],
                                         start=False, stop=True)
                        se_src = psum_se
                    elif cc < n_pe_se:
                        psum_se = pse.tile([P, KC], f32, tag="psum_se")
                        for c in range(NC_):
                            nc.tensor.matmul(psum_se[:], ident[:], ex[:, c, :],
                                             start=(c == 0), stop=(c == NC_ - 1))
                        se_src = psum_se
                    else:
                        s32 = tp.tile([P, 32, KC], bf16, tag="s32")
                        nc.vector.tensor_tensor(s32[:], ex[:, 0:32, :], ex[:, 32:64, :], Alu.add)
                        s16 = tp.tile([P, 16, KC], bf16, tag="s16")
                        nc.vector.tensor_tensor(s16[:], s32[:, 0:16, :], s32[:, 16:32, :], Alu.add)
                        s8 = tp.tile([P, 8, KC], bf16, tag="s8")
                        nc.vector.tensor_tensor(s8[:], s16[:, 0:8, :], s16[:, 8:16, :], Alu.add)
                        s4 = tp.tile([P, 4, KC], bf16, tag="s4")
                        nc.vector.tensor_tensor(s4[:], s8[:, 0:4, :], s8[:, 4:8, :], Alu.add)
                        s2 = tp.tile([P, 2, KC], bf16, tag="s2")
                        nc.vector.tensor_tensor(s2[:], s4[:, 0:2, :], s4[:, 2:4, :], Alu.add)
                        s1 = tp.tile([P, 1, KC], bf16, tag="s1")
                        nc.vector.tensor_tensor(s1[:], s2[:, 0:1, :], s2[:, 1:2, :], Alu.add)
                        se_sb = tp.tile([P, KC], f32, tag="se_sb")
                        nc.vector.tensor_tensor(se_sb[:, :, None], s1[:],
                                                ex[:, 64:65, :], Alu.add)
                        se_src = se_sb

                    lse = tp.tile([P, KC], f32, tag="lse")
                    nc.scalar.activation(lse[:], se_src[:], Act.Ln)
                    junk = tp.tile([P, KC], f32, tag="junk")
                    nc.vector.tensor_tensor(junk[:], lse[:],
                                            vt[:, cc * KC:(cc + 1) * KC], Alu.mult)
                    nc.vector.tensor_reduce(lse_sb[:, cc:cc + 1], junk[:],
                                            Ax.X, Alu.add)

                    # per-seq pred histograms
                    psum_h = ph.tile([2, NC_], f32, tag="psum_h")
                    for k in range(KC):
                        nc.tensor.matmul(psum_h[:], mbt[:, cc, k, :], eq[:, :, k],
                                         start=(k == 0), stop=(k == KC - 1))
                    nc.scalar.copy(hist_sb[:, cc, :], psum_h[:])

                nc.sync.dma_start(out=lse_out[:], in_=lse_sb[:])
                nc.sync.dma_start(out=hist_out[:], in_=hist_sb[:])
                nc.sync.dma_start(out=gps_out[:], in_=gps_sb[:])

            if repeat == 1:
                body()
            else:
                with tc.For_i(0, repeat, 1) as _i:
                    body(_i)

    nc.finalize()
    return nc


def _get_nc():
    if "nc" not in _BASS_CACHE:
        _BASS_CACHE["nc"] = _build_bass()
    return _BASS_CACHE["nc"]


def _seq_rscu_from_hist(counts, obs_counts_pos):
    """counts: [65] valid-codon counts; observed flag from aa-masked counts."""
    observed = (obs_counts_pos > 0) & IS_CODING
    obs_counts = counts * observed
    group_sum = np.zeros(NG, np.float64)
    np.add.at(group_sum, GROUP_IDS, obs_counts)
    tot = group_sum[GROUP_IDS]
    return np.where(observed & (tot > 0), obs_counts * NSYN / np.maximum(tot, 1.0), 0.0)


def _prep_in_maps(logits, gc_pred, pause_prob, m_f, maa_f, v_f):
    """Host-side shard + permute + cast. All device DMAs become contiguous."""
    # [64, 8192, 65] -> [64, 128(p), 64(k), 65(c)] -> [64, 128, 65, 64] bf16
    xall = np.ascontiguousarray(
        logits.reshape(B, P, KC, NC_).transpose(0, 1, 3, 2)).astype(BF16)
    mkk = np.stack([m_f.reshape(B, P, KC), maa_f.reshape(B, P, KC)],
                   axis=-1).astype(BF16)            # [64, 128, 64, 2]
    vkk = v_f.reshape(B, P, KC).astype(BF16)        # [64, 128, 64]
    gkk = np.stack([gc_pred.reshape(B, P, KC),
                    pause_prob.reshape(B, P, KC)], axis=0)  # [2, 64, 128, 64]
    ident = np.eye(P, dtype=BF16)
    identf8 = np.eye(P, dtype=ml_dtypes.float8_e4m3)
    tok = np.zeros((16, 16), np.float32)

    in_maps = []
    for c in range(NCORES):
        s0, s1 = c * SEQ_PER_CORE, (c + 1) * SEQ_PER_CORE
        in_maps.append({
            # [8, 128, 65, 64] -> [128, 8, 4160]
            "xb": np.ascontiguousarray(
                xall[s0:s1].transpose(1, 0, 2, 3).reshape(P, NCHUNK, CW)),
            # [8, 128, 64, 2] -> [128, 8, 64, 2]
            "mb": np.ascontiguousarray(mkk[s0:s1].transpose(1, 0, 2, 3)),
            # [8, 128, 64] -> [128, 8*64]
            "vt": np.ascontiguousarray(
                vkk[s0:s1].transpose(1, 0, 2).reshape(P, NCHUNK * KC)),
            # [2, 8, 128, 64] -> [128, 2, 8, 64] -> [128, 1024]
            "gpp": np.ascontiguousarray(
                gkk[:, s0:s1].transpose(2, 0, 1, 3).reshape(P, 2 * NCHUNK * KC)
            ).astype(np.float32),
            "ident": ident,
            "identf8": identf8,
            "tok": tok,
        })
    return in_maps


def kernel(logits, weight_matrix, ref_distributions, gc_pred, mfe, pause_prob,
           target_codon_ids, aa_ids, species_ids, mask):
    logits = np.ascontiguousarray(np.asarray(logits, np.float32))
    weight_matrix = np.asarray(weight_matrix, np.float32)
    ref_distributions = np.asarray(ref_distributions, np.float32)
    gc_pred = np.asarray(gc_pred, np.float32)
    mfe = np.asarray(mfe, np.float32)
    pause_prob = np.asarray(pause_prob, np.float32)
    t_ids = np.asarray(target_codon_ids).astype(np.int64)
    aa = np.asarray(aa_ids).astype(np.int64)
    sp = np.asarray(species_ids).astype(np.int64)
    msk = np.asarray(mask).astype(bool)

    m_f = msk.astype(np.float32)
    maa_f = (msk & (aa > 2)).astype(np.float32)
    v_b = t_ids != 0
    v_f = v_b.astype(np.float32)

    in_maps = _prep_in_maps(logits, gc_pred, pause_prob, m_f, maa_f, v_f)

    from concourse.bass_utils import run_bass_kernel_spmd
    nc = _get_nc()
    outs = None
    for _attempt in range(3):
        res = run_bass_kernel_spmd(nc, in_maps, core_ids=list(range(NCORES)))
        outs = res.results
        ok = all(
            np.isfinite(np.asarray(o[name], np.float64)).all()
            for o in outs for name in ("lse_acc", "hist", "gps"))
        if ok:
            break
    assert outs is not None

    # ---------------- host finalization ----------------
    # CE: sum(v*lse) from device; sum(v*x_t) exact gather on host
    lse_sum = sum(float(o["lse_acc"].astype(np.float64).sum()) for o in outs)
    x_t = np.take_along_axis(logits, t_ids[..., None].astype(np.int64),
                             axis=-1)[..., 0]
    xt_sum = float((x_t.astype(np.float64) * v_f).sum())
    v_count = float(v_f.sum())
    ce = (lse_sum - xt_sum) / max(v_count, 1.0)

    # pred histograms from device: [2, 8, 65] per core
    hist_m = np.concatenate([o["hist"][0] for o in outs], axis=0)   # [64, 65]
    hist_aa = np.concatenate([o["hist"][1] for o in outs], axis=0)  # [64, 65]

    # target-side histograms (host, exact)
    mask_cnt = m_f.sum(1)
    th_m = np.zeros((B, NC_), np.float64)
    th_aa = np.zeros((B, NC_), np.float64)
    for b in range(B):
        th_m[b] = np.bincount(t_ids[b], weights=m_f[b], minlength=NC_)
        th_aa[b] = np.bincount(t_ids[b], weights=maa_f[b], minlength=NC_)

    logw = np.log(np.maximum(weight_matrix, EPS)).astype(np.float64)  # [5, 65]

    def cai(hm):
        mean_log = (hm * logw[sp]).sum(1) / np.maximum(mask_cnt, 1.0)
        return np.exp(mean_log)

    pred_cai = cai(hist_m.astype(np.float64))
    target_cai = cai(th_m)
    cai_loss = np.maximum(target_cai - pred_cai, 0.0).mean()

    # RSCU KL per sequence
    kls = np.zeros(B, np.float64)
    for b in range(B):
        pc = hist_m[b].astype(np.float64).copy()
        pc[0] = 0.0
        pred_rscu = _seq_rscu_from_hist(pc, hist_aa[b])
        tc_ = th_m[b].copy()
        tc_[0] = 0.0
        target_rscu = _seq_rscu_from_hist(tc_, th_aa[b])
        combined = (0.7 * target_rscu
                    + 0.3 * ref_distributions[sp[b]].astype(np.float64) + EPS)
        pred = pred_rscu + EPS
        p_ = pred / pred.sum()
        t_ = combined / combined.sum()
        kls[b] = (t_ * (np.log(t_) - np.log(p_))).sum()
    rscu_loss = kls.mean()

    # gc / dynamics from device per-(partition, seq) sums
    gps = np.stack([o["gps"].reshape(P, 2, NCHUNK) for o in outs])  # [8,128,2,8]
    seq_sums = gps.astype(np.float64).sum(1)                        # [8, 2, 8]
    gc_means = seq_sums[:, 0, :].reshape(-1) / L
    pp_means = seq_sums[:, 1, :].reshape(-1) / L
    gc_loss = ((gc_means - 0.5) ** 2).mean()
    dynamics_loss = ((pp_means - 0.1) ** 2).mean()
    structure_loss = float(((mfe.astype(np.float64) + 20.0) ** 2).mean())

    total = (LOSS_W["ce"] * ce + LOSS_W["cai"] * cai_loss
             + LOSS_W["rscu"] * rscu_loss + LOSS_W["gc"] * gc_loss
             + LOSS_W["structure"] * structure_loss
             + LOSS_W["dynamics"] * dynamics_loss)
    return np.float32(total)
